# revision 1
# baseline (speedup 1.0000x reference)
"""Trainium2 Bass kernel for a 3x3 stride-1 pad-1 Conv2d (NCHW).

Problem (hardcoded): x (16, 128, 128, 128) f32, K (3, 3, 128, 256) f32.
The reference reinterprets K's flat buffer as (Cin, kh, kw, Cout) and only
writes output rows/cols 0..124 (the rest of the 128x128 output stays zero).

Strategy: data-parallel over batch — 2 images per NeuronCore on 8 cores.
Per image the padded activation plane (Cin=128 partitions x 130x130) lives
in SBUF; the conv is 9 accumulated matmuls (contraction over Cin=128) per
output tile of 4 rows x 128 cols (N=512, one PSUM bank) per Cout half.
Matmuls run in float32r (TF32-like full-rate PE path; host pre-rounds the
inputs). Only the valid 125x125 region is DMA'd out; the host zeroes the
border strips.
"""

import numpy as np

import concourse.bacc as bacc
import concourse.mybir as mybir
import concourse.tile as tile
from concourse.bass_utils import run_bass_kernel_spmd

N_CORES = 8
B, CIN, H, W = 16, 128, 128, 128
COUT = 256
BPC = B // N_CORES  # images per core
HP, WP = H + 2, W + 2  # zero-padded plane
VALID = 125  # valid output rows/cols; rest is zero
ROWS_PER_TILE = 4  # 4 rows x 128 cols = 512 = one PSUM bank
F32 = mybir.dt.float32
F32R = mybir.dt.float32r

_NC_CACHE = {}


def _build_nc(reps=1):
    nc = bacc.Bacc()
    # Inputs are declared float32r (TF32-like: fp32 with 11 mantissa bits,
    # low 12 bits zero). Host pre-rounds, so DMA'd bytes are valid fp32r.
    # x arrives pre-padded (130x130, zero borders): fp32r is an opt-in ISA
    # dtype that DVE memset doesn't support, so padding happens on host.
    x_in = nc.dram_tensor("x", [BPC, CIN, HP, WP], F32R, kind="ExternalInput")
    w_in = nc.dram_tensor("w", [CIN, 9 * COUT], F32R, kind="ExternalInput")
    out_t = nc.dram_tensor("out", [BPC, COUT, H, W], F32, kind="ExternalOutput")

    with tile.TileContext(nc) as tc:
        with (
            tc.tile_pool(name="wpool", bufs=1) as wpool,
            tc.tile_pool(name="xpool", bufs=2) as xpool,
            tc.tile_pool(name="opool", bufs=6) as opool,
            tc.tile_pool(name="pspool", bufs=8, space="PSUM") as pspool,
        ):
            w_sb = wpool.tile([CIN, 9 * COUT], F32R)
            nc.sync.dma_start(out=w_sb[:], in_=w_in[:])

            for b in [b for _ in range(reps) for b in range(BPC)]:
                x_pad = xpool.tile([CIN, HP, WP], F32R)
                nc.sync.dma_start(out=x_pad[:], in_=x_in[b])

                for rb in range(H // ROWS_PER_TILE):
                    r = rb * ROWS_PER_TILE
                    vr = min(ROWS_PER_TILE, VALID - r)
                    if vr <= 0:
                        continue
                    for c2 in range(2):
                        ps = pspool.tile([128, ROWS_PER_TILE, W], F32)
                        for i, t in enumerate(range(9)):
                            kh, kw = divmod(t, 3)
                            c0 = t * COUT + c2 * 128
                            lhsT = w_sb[:, c0 : c0 + 128]
                            rhs = x_pad[:, r + kh : r + kh + ROWS_PER_TILE, kw : kw + W]
                            nc.tensor.matmul(
                                ps[:],
                                lhsT,
                                rhs,
                                start=(i == 0),
                                stop=(i == 8),
                            )
                        ob = opool.tile([128, ROWS_PER_TILE, W], F32)
                        nc.vector.tensor_copy(out=ob[:], in_=ps[:])
                        nc.sync.dma_start(
                            out=out_t[b, c2 * 128 : (c2 + 1) * 128, r : r + vr, 0:VALID],
                            in_=ob[:, 0:vr, 0:VALID],
                        )
    # Bacc defers register allocation and wait-splitting to compile(),
    # which finalize() runs; the SPMD exec path expects it done already.
    nc.finalize()
    return nc


def _get_nc(reps=1):
    if reps not in _NC_CACHE:
        _NC_CACHE[reps] = _build_nc(reps)
    return _NC_CACHE[reps]


def _round_fp32r(a):
    """Round fp32 to the hardware fp32r format: 11 mantissa bits, RNE."""
    u = np.ascontiguousarray(a, dtype=np.float32).view(np.uint32)
    r = (u + np.uint32(0x7FF) + ((u >> np.uint32(12)) & np.uint32(1))) & np.uint32(
        0xFFFFF000
    )
    return r.view(np.float32)


def _run(x, K, trace=False, reps=1):
    x_pad = np.zeros((B, CIN, HP, WP), dtype=np.float32)
    x_pad[:, :, 1 : H + 1, 1 : W + 1] = _round_fp32r(x)
    # Reference reinterprets K's flat buffer as (Cin, kh, kw, Cout); flat
    # (128, 2304) rows are Cin, cols are (kh*3+kw)*256 + cout.
    w_host = _round_fp32r(np.asarray(K, dtype=np.float32)).reshape(CIN, 9 * COUT)
    in_maps = [
        {"x": x_pad[i * BPC : (i + 1) * BPC], "w": w_host} for i in range(N_CORES)
    ]
    res = run_bass_kernel_spmd(
        _get_nc(reps), in_maps, list(range(N_CORES)), trace=trace
    )
    out = np.concatenate([res.results[i]["out"] for i in range(N_CORES)], axis=0)
    # Device only writes the valid 125x125 region; zero the border strips.
    out[:, :, VALID:, :] = 0
    out[:, :, :, VALID:] = 0
    return out, res


def kernel(x, K):
    out, _ = _run(x, K, trace=False)
    return out



# revision 4
# speedup vs baseline: 1.1897x; 1.1897x over previous
"""Trainium2 Bass kernel for a 3x3 stride-1 pad-1 Conv2d (NCHW).

Problem (hardcoded): x (16, 128, 128, 128) f32, K (3, 3, 128, 256) f32.
The reference reinterprets K's flat buffer as (Cin, kh, kw, Cout) and only
writes output rows/cols 0..124 (the rest of the 128x128 output stays zero).

Strategy: data-parallel over batch — 2 images per NeuronCore on 8 cores.
All device-side data is float16 (host pre-rounds; quantization ~2.5e-4 rel,
PSUM accumulation stays fp32). Per image the padded activation plane is
streamed in 8 chunks of 18 rows (16 output rows + 2 conv halo) so the first
matmul fires ~10us in instead of waiting ~30us for a whole-image DMA. The
conv is 9 accumulated matmuls (contraction over Cin=128) per output tile of
4 rows x 128 cols (N=512, one PSUM bank) per Cout half. fp16 weights get the
compiler's fast-weight-load path, hiding LDWEIGHTS under the matmul stream.
Outputs are staged per-chunk in SBUF (16 rows x 128 cols x fp16) and written
with one contiguous-4KB-per-partition DMA per (chunk, cout-half) on the
scalar-engine HWDGE ring, overlapping the input ring. Rows/cols >= 125 are
zeroed on host. A short burst of dummy matmuls warms the PE HAM clock gate
(1.2 -> 2.4 GHz) while the first chunk DMA is in flight.
"""

import numpy as np

import concourse.bacc as bacc
import concourse.mybir as mybir
import concourse.tile as tile
from concourse.bass_utils import run_bass_kernel_spmd

N_CORES = 8
B, CIN, H, W = 16, 128, 128, 128
COUT = 256
BPC = B // N_CORES  # images per core
HP, WP = H + 2, W + 2  # zero-padded plane
VALID = 125  # valid output rows/cols; rest is zero
NCHUNK = 8
CHUNK_ROWS = 16  # output rows per chunk
CHUNK_IN = CHUNK_ROWS + 2  # input rows per chunk (conv halo)
F32 = mybir.dt.float32
F16 = mybir.dt.float16

_NC_CACHE = {}


def _build_nc(reps=1):
    nc = bacc.Bacc()
    x_in = nc.dram_tensor("x", [BPC, CIN, HP, WP], F16, kind="ExternalInput")
    # Reference reinterprets K's flat buffer as (Cin, kh, kw, Cout); host
    # ships it as [Cin, (kh*3+kw)*256 + cout].
    w_in = nc.dram_tensor("w", [CIN, 9 * COUT], F16, kind="ExternalInput")
    out_t = nc.dram_tensor("out", [BPC, COUT, H, W], F16, kind="ExternalOutput")

    with tile.TileContext(nc) as tc:
        with (
            tc.tile_pool(name="wpool", bufs=1) as wpool,
            tc.tile_pool(name="xpool", bufs=6) as xpool,
            tc.tile_pool(name="opool", bufs=4) as opool,
            tc.tile_pool(name="pspool", bufs=8, space="PSUM") as pspool,
        ):
            # PE warmup: dummy matmuls with no DMA dependency keep the PE
            # busy through one HAM activity window so the real stream starts
            # at 2.4 GHz instead of paying the 1.2 GHz cold ramp.
            dummy = wpool.tile([CIN, 256], F16)
            nc.gpsimd.memset(dummy[:], 0.0)
            # Shares the "ps" tag (and thus slot set) with the conv tiles.
            wps = pspool.tile([128, 4, W], F32, name="ps")
            for _ in range(30):
                nc.tensor.matmul(
                    wps[:, 0:1, :],
                    dummy[:, 0:128],
                    dummy[:, 128:256],
                    start=True,
                    stop=True,
                )

            # Weights on the scalar-engine HWDGE ring, chunk loads on the
            # sync-engine ring: the two transfers overlap.
            w_sb = wpool.tile([CIN, 9 * COUT], F16)
            nc.scalar.dma_start(out=w_sb[:], in_=w_in[:])

            for b in [b for _ in range(reps) for b in range(BPC)]:
                for c in range(NCHUNK):
                    xc = xpool.tile([CIN, CHUNK_IN, WP], F16)
                    nc.sync.dma_start(
                        out=xc[:],
                        in_=x_in[b, :, 16 * c : 16 * c + CHUNK_IN, :],
                    )
                    last = c == NCHUNK - 1
                    ocs = [
                        opool.tile([128, CHUNK_ROWS, W], F16, name=f"oc{c2}")
                        for c2 in range(2)
                    ]
                    for rb in range(4):
                        r0 = 4 * rb  # chunk-local output row
                        # Global output rows 125..127 are never read; the
                        # final row-block computes only its single valid row.
                        nrows = 1 if last and rb == 3 else 4
                        for c2 in range(2):
                            ps = pspool.tile([128, nrows, W], F32)
                            for t in range(9):
                                kh, kw = divmod(t, 3)
                                c0 = t * COUT + c2 * 128
                                nc.tensor.matmul(
                                    ps[:],
                                    w_sb[:, c0 : c0 + 128],
                                    xc[:, r0 + kh : r0 + kh + nrows, kw : kw + W],
                                    start=(t == 0),
                                    stop=(t == 8),
                                )
                            nc.vector.tensor_copy(
                                out=ocs[c2][:, r0 : r0 + nrows, :], in_=ps[:]
                            )
                    out_rows = 13 if last else CHUNK_ROWS  # rows 112..124
                    for c2 in range(2):
                        nc.scalar.dma_start(
                            out=out_t[
                                b,
                                c2 * 128 : (c2 + 1) * 128,
                                16 * c : 16 * c + out_rows,
                                :,
                            ],
                            in_=ocs[c2][:, 0:out_rows, :],
                        )
    # Bacc defers register allocation and wait-splitting to compile(),
    # which finalize() runs; the SPMD exec path expects it done already.
    nc.finalize()
    return nc


def _get_nc(reps=1):
    if reps not in _NC_CACHE:
        _NC_CACHE[reps] = _build_nc(reps)
    return _NC_CACHE[reps]


def _run(x, K, trace=False, reps=1):
    x_pad = np.zeros((B, CIN, HP, WP), dtype=np.float16)
    x_pad[:, :, 1 : H + 1, 1 : W + 1] = np.asarray(x, dtype=np.float32).astype(
        np.float16
    )
    # Reference reinterprets K's flat buffer as (Cin, kh, kw, Cout); flat
    # (128, 2304) rows are Cin, cols are (kh*3+kw)*256 + cout.
    w_host = (
        np.asarray(K, dtype=np.float32).reshape(CIN, 9 * COUT).astype(np.float16)
    )
    in_maps = [
        {"x": x_pad[i * BPC : (i + 1) * BPC], "w": w_host} for i in range(N_CORES)
    ]
    res = run_bass_kernel_spmd(
        _get_nc(reps), in_maps, list(range(N_CORES)), trace=trace
    )
    out = np.concatenate(
        [res.results[i]["out"] for i in range(N_CORES)], axis=0
    ).astype(np.float32)
    # Device only writes valid rows; zero the border strips (rows/cols >=125).
    out[:, :, VALID:, :] = 0
    out[:, :, :, VALID:] = 0
    return out, res


def kernel(x, K):
    out, _ = _run(x, K, trace=False)
    return out


# revision 7
# speedup vs baseline: 1.2270x; 1.0313x over previous
"""Trainium2 Bass kernel for a 3x3 stride-1 pad-1 Conv2d (NCHW).

Problem (hardcoded): x (16, 128, 128, 128) f32, K (3, 3, 128, 256) f32.
The reference reinterprets K's flat buffer as (Cin, kh, kw, Cout) and only
writes output rows/cols 0..124 (the rest of the 128x128 output stays zero).

Strategy: data-parallel over batch — 2 images per NeuronCore on 8 cores.
All device-side data is float16 (host pre-rounds; quantization ~2.5e-4 rel,
PSUM accumulation stays fp32). Per image the padded activation plane is
streamed in 16 chunks of 10 rows (8 output rows + 2 conv halo) so the first
matmul fires ~12us in instead of waiting ~30us for a whole-image DMA. The
conv is 9 accumulated matmuls (contraction over Cin=128) per output tile of
4 rows x 125 valid cols (N=500, one PSUM bank) per Cout half; cols >= 125
of the output are never computed (host zeroes them). fp16 weights get the
compiler's fast-weight-load path, hiding LDWEIGHTS under the matmul stream.
Outputs are staged per-chunk in SBUF (8 rows x 128 cols x fp16) and written
with one contiguous-2KB-per-partition DMA per (chunk, cout-half) on the
scalar-engine HWDGE ring, overlapping the input ring; the last chunk issues
per-row-block DMAs so the final transfer is tiny. A burst of dummy matmuls
warms the PE HAM clock gate (1.2 -> 2.4 GHz) while the first chunk DMA is
in flight.
"""

import numpy as np

import concourse.bacc as bacc
import concourse.mybir as mybir
import concourse.tile as tile
from concourse.bass_utils import run_bass_kernel_spmd

N_CORES = 8
B, CIN, H, W = 16, 128, 128, 128
COUT = 256
BPC = B // N_CORES  # images per core
HP, WP = H + 2, W + 2  # zero-padded plane
VALID = 125  # valid output rows/cols; rest is zero
NCHUNK = 16
CHUNK_ROWS = 8  # output rows per chunk
CHUNK_IN = CHUNK_ROWS + 2  # input rows per chunk (conv halo)
F32 = mybir.dt.float32
F16 = mybir.dt.float16

_NC_CACHE = {}


def _build_nc(reps=1):
    nc = bacc.Bacc()
    x_in = nc.dram_tensor("x", [BPC, CIN, HP, WP], F16, kind="ExternalInput")
    # Reference reinterprets K's flat buffer as (Cin, kh, kw, Cout); host
    # ships it as [Cin, (kh*3+kw)*256 + cout].
    w_in = nc.dram_tensor("w", [CIN, 9 * COUT], F16, kind="ExternalInput")
    out_t = nc.dram_tensor("out", [BPC, COUT, H, W], F16, kind="ExternalOutput")

    with tile.TileContext(nc) as tc:
        with (
            tc.tile_pool(name="wpool", bufs=1) as wpool,
            tc.tile_pool(name="xpool", bufs=8) as xpool,
            tc.tile_pool(name="opool", bufs=4) as opool,
            tc.tile_pool(name="pspool", bufs=8, space="PSUM") as pspool,
        ):
            # PE warmup: dummy matmuls with no DMA dependency keep the PE
            # busy through the HAM activity window until the first chunk
            # lands, so the real stream starts at 2.4 GHz.
            dummy = wpool.tile([CIN, 256], F16)
            nc.gpsimd.memset(dummy[:], 0.0)
            # Shares the "ps" tag (and thus slot set) with the conv tiles.
            wps = pspool.tile([128, 4, VALID], F32, name="ps")
            for _ in range(48):
                nc.tensor.matmul(
                    wps[:, 0:1, :],
                    dummy[:, 0:128],
                    dummy[:, 128 : 128 + VALID],
                    start=True,
                    stop=True,
                )

            # Weights on the scalar-engine HWDGE ring, chunk loads on the
            # sync-engine ring: the two transfers overlap.
            w_sb = wpool.tile([CIN, 9 * COUT], F16)
            nc.scalar.dma_start(out=w_sb[:], in_=w_in[:])

            for b in [b for _ in range(reps) for b in range(BPC)]:
                for c in range(NCHUNK):
                    xc = xpool.tile([CIN, CHUNK_IN, WP], F16)
                    nc.sync.dma_start(
                        out=xc[:],
                        in_=x_in[b, :, CHUNK_ROWS * c : CHUNK_ROWS * c + CHUNK_IN, :],
                    )
                    last = c == NCHUNK - 1
                    ocs = [
                        opool.tile([128, CHUNK_ROWS, W], F16, name=f"oc{c2}")
                        for c2 in range(2)
                    ]
                    if not last:
                        # Cols 125..127 are never computed but ride along in
                        # the contiguous store; zero them (host re-zeroes).
                        for oc in ocs:
                            nc.gpsimd.memset(oc[:, :, VALID:W], 0.0)
                    for rb in range(2):
                        r0 = 4 * rb  # chunk-local output row
                        # Global output rows 125..127 are never read; the
                        # final row-block computes only its single valid row.
                        nrows = 1 if last and rb == 1 else 4
                        for c2 in range(2):
                            ps = pspool.tile([128, nrows, VALID], F32)
                            for t in range(9):
                                kh, kw = divmod(t, 3)
                                c0 = t * COUT + c2 * 128
                                nc.tensor.matmul(
                                    ps[:],
                                    w_sb[:, c0 : c0 + 128],
                                    xc[:, r0 + kh : r0 + kh + nrows, kw : kw + VALID],
                                    start=(t == 0),
                                    stop=(t == 8),
                                )
                            nc.vector.tensor_copy(
                                out=ocs[c2][:, r0 : r0 + nrows, 0:VALID], in_=ps[:]
                            )
                            if last:
                                # Tiny per-row-block stores keep the final
                                # DMA (and its completion latency) short.
                                nc.scalar.dma_start(
                                    out=out_t[
                                        b,
                                        c2 * 128 : (c2 + 1) * 128,
                                        CHUNK_ROWS * c + r0 : CHUNK_ROWS * c
                                        + r0
                                        + nrows,
                                        0:VALID,
                                    ],
                                    in_=ocs[c2][:, r0 : r0 + nrows, 0:VALID],
                                )
                    if not last:
                        for c2 in range(2):
                            nc.scalar.dma_start(
                                out=out_t[
                                    b,
                                    c2 * 128 : (c2 + 1) * 128,
                                    CHUNK_ROWS * c : CHUNK_ROWS * (c + 1),
                                    :,
                                ],
                                in_=ocs[c2][:],
                            )
    # Bacc defers register allocation and wait-splitting to compile(),
    # which finalize() runs; the SPMD exec path expects it done already.
    nc.finalize()
    return nc


def _get_nc(reps=1):
    if reps not in _NC_CACHE:
        _NC_CACHE[reps] = _build_nc(reps)
    return _NC_CACHE[reps]


def _run(x, K, trace=False, reps=1):
    x_pad = np.zeros((B, CIN, HP, WP), dtype=np.float16)
    x_pad[:, :, 1 : H + 1, 1 : W + 1] = np.asarray(x, dtype=np.float32).astype(
        np.float16
    )
    # Reference reinterprets K's flat buffer as (Cin, kh, kw, Cout); flat
    # (128, 2304) rows are Cin, cols are (kh*3+kw)*256 + cout.
    w_host = (
        np.asarray(K, dtype=np.float32).reshape(CIN, 9 * COUT).astype(np.float16)
    )
    in_maps = [
        {"x": x_pad[i * BPC : (i + 1) * BPC], "w": w_host} for i in range(N_CORES)
    ]
    res = run_bass_kernel_spmd(
        _get_nc(reps), in_maps, list(range(N_CORES)), trace=trace
    )
    out = np.concatenate(
        [res.results[i]["out"] for i in range(N_CORES)], axis=0
    ).astype(np.float32)
    # Device only writes valid rows/cols 0..124; zero the border strips.
    out[:, :, VALID:, :] = 0
    out[:, :, :, VALID:] = 0
    return out, res


def kernel(x, K):
    out, _ = _run(x, K, trace=False)
    return out
